# revision 1
# baseline (speedup 1.0000x reference)
"""Trainium2 Bass kernel for fused segment-mean + linear projection.

Reference computation (for x[N,15], sorted batch[N] in [0,G), W[5,15], b[5]):
    sums[g]  = segment_sum(x, batch)          # [G, 15]
    counts[g]= segment_sum(1, batch)          # [G]
    mean     = sums / max(counts, 1)
    out      = where(counts > 0, mean @ W.T + b, 0)   # [G, 5]

Strategy (8 NeuronCores, data parallel over contiguous graph-id ranges):
  Host (index-only preprocessing, no arithmetic on x beyond repacking):
    - each core owns G/8 consecutive graphs; its nodes are repacked into
      "windows" of GPW=32 graphs x 8192 node-slots (4 chunks of 128
      16-node blocks), each graph zero-padded to a 16-node multiple so
      every block belongs to exactly one graph. Graphs that do not fit
      their window spill whole into a small per-stripe overflow stream.
    - all program shapes / the matmul schedule are data-independent, so
      one SPMD program serves all 8 cores; per-core data differs only in
      the input tables (packed x, block->graph assignment, 1/count, ...).
  Device (per core):
    - DMA the packed x stream block-interleaved (block t -> partition
      t%128, chunk t//128), strided DVE tensor_reduce -> per-block sums
      B[128, NCHUNK*15].
    - PE matmuls with on-device-built one-hot matrices (iota + is_equal
      against a tiny host table) scatter-add the 128 block-sums of each
      chunk into per-quadrant PSUM accumulators ([32 graphs, nstripe*15],
      one bank each; disjoint column writes, single start=True opener).
    - fused mean (multiply by 1/count), then a small DVE projection
      (mean @ W.T + b, empty-graph masking) -> out [G/8, 5].
  Host: concatenate the 8 core outputs.
"""

import sys

for _p in ("/opt/trn_rl_repo",):
    if _p not in sys.path:
        sys.path.insert(0, _p)

import numpy as np
from contextlib import ExitStack

import concourse.bass as bass
import concourse.bacc as bacc
import concourse.tile as tile
from concourse import mybir
from concourse.bass_utils import run_bass_kernel_spmd

P = 128          # partitions
BLK = 16         # nodes per block
D = 15           # feature dim
O = 5            # output dim
GPW = 32         # graphs per window
CPW = 4          # chunks per window (chunk = 128 blocks = 2048 node slots)
SLOTS_W = CPW * P * BLK  # 4096 node slots per window

F32 = mybir.dt.float32


# ----------------------------------------------------------------------------
# host planner
# ----------------------------------------------------------------------------

class Plan:
    """Per-run packing plan. All *shape* fields are uniform across cores."""

    def __init__(self, batch, n_cores, G, W=None, b=None):
        self.W = (np.zeros((O, D), np.float32) if W is None
                  else np.asarray(W, np.float32))
        self.b = (np.zeros(O, np.float32) if b is None
                  else np.asarray(b, np.float32))
        batch = np.asarray(batch)
        N = batch.shape[0]
        assert G % (n_cores * P) == 0
        self.G = G
        self.n_cores = n_cores
        self.gpc = G // n_cores                  # graphs per core
        self.nwin = self.gpc // GPW              # windows per core
        self.nstripe = self.gpc // P             # psum stripes per core
        self.nchunk = self.nwin * CPW            # main-stream chunks per core
        self.lslots = self.nwin * SLOTS_W        # node slots per core
        assert self.nwin % self.nstripe == 0
        self.wps = self.nwin // self.nstripe     # windows per stripe (8)

        bounds = np.searchsorted(batch, np.arange(G + 1))
        counts = np.diff(bounds).astype(np.int64)
        self.counts = counts
        self.inv = (1.0 / np.maximum(counts, 1.0)).astype(np.float32)
        self.nonempty = (counts > 0).astype(np.float32)

        nblk_g = (counts + BLK - 1) // BLK       # blocks per graph

        # ---- window placement (per core) ----
        # placements[c] = list of (graph, slot_base_in_core) for windowed graphs
        # overflow[c][stripe] = list of graphs spilled to that stripe's stream
        self.placements = []
        self.overflow = []
        max_oslots = 8 * BLK  # overflow slots per stripe (uniform; >= actual max)
        for c in range(n_cores):
            g0 = c * self.gpc
            placed = []
            oflow = [[] for _ in range(self.nstripe)]
            for w in range(self.nwin):
                pos = 0
                for j in range(GPW):
                    g = g0 + w * GPW + j
                    need = int(nblk_g[g]) * BLK
                    if need == 0:
                        continue
                    if pos + need <= SLOTS_W:
                        placed.append((g, w * SLOTS_W + pos))
                        pos += need
                    else:
                        oflow[w // self.wps].append(g)
            for s in range(self.nstripe):
                used = sum(int(nblk_g[g]) * BLK for g in oflow[s])
                max_oslots = max(max_oslots, used)
            self.placements.append(placed)
            self.overflow.append(oflow)

        # overflow blocks per stripe: a single partial chunk (K<128 matmuls)
        # when it fits, whole chunks otherwise
        max_oblk = -(-max_oslots // BLK)
        if max_oblk <= P:
            self.osb = max(8, -(-max_oblk // 8) * 8)
            self.ocps = 1
        else:
            self.osb = P
            self.ocps = -(-max_oblk // P)
        self.osps = self.ocps * self.osb * BLK   # overflow slots per stripe
        self.noch = self.nstripe * self.ocps     # total overflow chunks
        self.bounds = bounds
        self.N = N

    def core_tables(self, c, x):
        """Build per-core input arrays. x is the full [N, D] float32 array."""
        lslots, nchunk = self.lslots, self.nchunk
        g0 = c * self.gpc

        idx = np.full(lslots, -1, dtype=np.int64)
        asg = np.full(lslots // BLK, -1.0, dtype=np.float32)  # local graph per block
        for g, base in self.placements[c]:
            s0, cnt = int(self.bounds[g]), int(self.counts[g])
            idx[base : base + cnt] = np.arange(s0, s0 + cnt)
            nb = (cnt + BLK - 1) // BLK
            asg[base // BLK : base // BLK + nb] = g - g0

        # overflow stream: per stripe a fixed region of osps slots
        oidx = np.full(self.nstripe * self.osps, -1, dtype=np.int64)
        oasg = np.full(self.nstripe * self.osps // BLK, -1.0, dtype=np.float32)
        for s in range(self.nstripe):
            pos = s * self.osps
            for g in self.overflow[c][s]:
                s0, cnt = int(self.bounds[g]), int(self.counts[g])
                nb = (cnt + BLK - 1) // BLK
                assert pos + nb * BLK <= (s + 1) * self.osps, "overflow overrun"
                oidx[pos : pos + cnt] = np.arange(s0, s0 + cnt)
                oasg[pos // BLK : pos // BLK + nb] = (g - g0) - s * P
                pos += nb * BLK

        def pack(idx_arr):
            out = x[np.clip(idx_arr, 0, self.N - 1)]
            out[idx_arr < 0] = 0.0
            return np.ascontiguousarray(out, dtype=np.float32)

        xw = pack(idx)                            # [lslots, D]
        xb = pack(oidx)                           # [nstripe*osps, D]

        # block t -> partition t%128, chunk t//128; window of chunk m = m//CPW
        t = np.arange(lslots // BLK)
        asgJ = np.full((P, nchunk), -1.0, dtype=np.float32)
        win_base = (t // P // CPW) * GPW
        vals = np.where(asg >= 0, asg - win_base, -1.0)
        asgJ[t % P, t // P] = vals

        asgO = np.full((P, self.noch), -1.0, dtype=np.float32)
        asgO[: self.osb, :] = oasg.reshape(self.noch, self.osb).T

        def stripe_pack(v):
            # graph g (local) -> [partition g%128, col g//128]
            return np.ascontiguousarray(
                v[g0 : g0 + self.gpc].reshape(self.nstripe, P).T.astype(np.float32)
            )

        # fold 1/count, W, b and the empty-graph mask into two tables:
        #   winv[p, o, s, f] = W[o, f] * inv[g(p, s)]
        #   bne[p, s, o]     = b[o] * nonempty[g(p, s)]
        inv_ps = stripe_pack(self.inv)                       # [P, nstripe]
        ne_ps = stripe_pack(self.nonempty)                   # [P, nstripe]
        winv = (inv_ps[:, None, :, None] *
                self.W[None, :, None, :]).astype(np.float32)  # [P,O,S,D]
        bne = (ne_ps[:, :, None] * self.b[None, None, :]).astype(np.float32)

        return {
            "xw": xw.reshape(-1),
            "xb": xb.reshape(-1),
            "asgJ": asgJ,
            "asgO": asgO,
            "winv": np.ascontiguousarray(winv.reshape(P, -1)),
            "bne": np.ascontiguousarray(bne.reshape(P, -1)),
        }


# ----------------------------------------------------------------------------
# device program
# ----------------------------------------------------------------------------

def build_program(plan, W, b):
    """Build + compile the SPMD Bass program (one program, 8 cores)."""
    nchunk, noch, nstripe = plan.nchunk, plan.noch, plan.nstripe
    lslots = plan.lslots
    wps = plan.wps

    nc = bacc.Bacc("TRN2", target_bir_lowering=False, debug=False)

    xw = nc.dram_tensor("xw", [lslots * D], F32, kind="ExternalInput")
    xb = nc.dram_tensor("xb", [nstripe * plan.osps * D], F32, kind="ExternalInput")
    asgJ = nc.dram_tensor("asgJ", [P, nchunk], F32, kind="ExternalInput")
    asgO = nc.dram_tensor("asgO", [P, noch], F32, kind="ExternalInput")
    winv_t = nc.dram_tensor("winv", [P, O * nstripe * D], F32, kind="ExternalInput")
    bne_t = nc.dram_tensor("bne", [P, nstripe * O], F32, kind="ExternalInput")
    out_t = nc.dram_tensor("out", [plan.gpc * O], F32, kind="ExternalOutput")

    CB = 240  # elements per block (BLK * D)
    # x tiles: chunks per DMA tile. Tapered: small first tile so DVE starts
    # early, small last tiles so the post-DMA tail (reduce+route+proj) is short.
    KCS = []
    rem = nchunk
    KCS.append(min(8, rem)); rem -= KCS[-1]
    while rem - 56 >= 32:
        KCS.append(32); rem -= 32
    while rem > 16:
        KCS.append(min(16, rem)); rem -= KCS[-1]
    while rem > 0:
        KCS.append(min(8, rem)); rem -= KCS[-1]
    assert sum(KCS) == nchunk

    with tile.TileContext(nc) as tc, ExitStack() as ctx:
        consts = ctx.enter_context(tc.tile_pool(name="consts", bufs=1))
        xpool = ctx.enter_context(tc.tile_pool(name="xpool", bufs=3))
        bpool = ctx.enter_context(tc.tile_pool(name="bpool", bufs=1))
        ppool = ctx.enter_context(tc.tile_pool(name="ppool", bufs=1, space="PSUM"))

        def ap_of(handle, offset, pattern):
            return bass.AP(tensor=handle.ap().tensor, offset=offset, ap=pattern)

        # ---- constant tables (ACT HWDGE ring; keeps SP ring free for x tiles) ----
        asgJ_sb = consts.tile([P, nchunk], F32)
        nc.scalar.dma_start(out=asgJ_sb[:], in_=asgJ.ap())
        asgO_sb = consts.tile([P, noch], F32)
        nc.scalar.dma_start(out=asgO_sb[:], in_=asgO.ap())
        winv_sb = consts.tile([P, O * nstripe * D], F32)
        bne_sb = consts.tile([P, nstripe * O], F32)

        # ---- iota rows for one-hot construction ----
        iota_w = consts.tile([P, GPW], F32)
        nc.gpsimd.iota(
            iota_w[:],
            pattern=[[1, GPW]],
            base=0,
            channel_multiplier=0,
            allow_small_or_imprecise_dtypes=True,
        )
        iota_o = consts.tile([P, P], F32)
        nc.gpsimd.iota(
            iota_o[:],
            pattern=[[1, P]],
            base=0,
            channel_multiplier=0,
            allow_small_or_imprecise_dtypes=True,
        )
        # identity selection matrix for the quadrant recombine:
        # i4_sb[k, q*P + m] = 1.0 iff m == q*GPW + k
        i4_sb = consts.tile([GPW, (P // GPW) * P], F32)
        nc.gpsimd.memset(i4_sb[:], 0.0)
        nc.gpsimd.affine_select(
            out=i4_sb[:],
            in_=i4_sb[:],
            compare_op=mybir.AluOpType.not_equal,
            fill=1.0,
            base=0,
            channel_multiplier=-1,
            pattern=[[-GPW, P // GPW], [1, P]],
        )
        # one-hot arenas:
        #   onehot[p, m*GPW + w] = (asgJ[p, m] == w)   built per-stripe, DVE,
        #     interleaved into the reduce stream (see emit_oh below)
        #   oneO[p, ch*P + w]   = (asgO[p, ch] == w)   one small DVE op
        onehot = bpool.tile([P, nchunk * GPW], F32)
        oneO = bpool.tile([P, noch * P], F32)
        cps = nchunk // nstripe  # main-stream chunks per stripe

        def emit_oh(s):
            return nc.vector.tensor_tensor(
                out=bass.AP(
                    tensor=onehot.tensor, offset=onehot.offset + s * cps * GPW,
                    ap=[onehot.ap[0], [GPW, cps], [1, GPW]],
                ),
                in0=bass.AP(
                    tensor=asgJ_sb.tensor, offset=asgJ_sb.offset + s * cps,
                    ap=[asgJ_sb.ap[0], [1, cps], [0, GPW]],
                ),
                in1=bass.AP(
                    tensor=iota_w.tensor, offset=iota_w.offset,
                    ap=[iota_w.ap[0], [0, cps], [1, GPW]],
                ),
                op=mybir.AluOpType.is_equal,
            )

        def emit_oo():
            return nc.vector.tensor_tensor(
                out=bass.AP(
                    tensor=oneO.tensor, offset=oneO.offset,
                    ap=[oneO.ap[0], [P, noch], [1, P]],
                ),
                in0=bass.AP(
                    tensor=asgO_sb.tensor, offset=asgO_sb.offset,
                    ap=[asgO_sb.ap[0], [1, noch], [0, P]],
                ),
                in1=bass.AP(
                    tensor=iota_o.tensor, offset=iota_o.offset,
                    ap=[iota_o.ap[0], [0, noch], [1, P]],
                ),
                op=mybir.AluOpType.is_equal,
            )

        # ---- overflow stream: load (reduce emitted after first x reduces) ----
        osb = plan.osb
        xb_sb = bpool.tile([P, noch * CB], F32)
        nc.scalar.dma_start(
            out=xb_sb[:osb, :],
            in_=ap_of(xb, 0, [[CB, osb], [CB * osb, noch], [1, CB]]),
        )
        Bo = bpool.tile([P, noch * D], F32)

        def emit_bo_reduce():
            return nc.vector.tensor_reduce(
                out=bass.AP(
                    tensor=Bo.tensor, offset=Bo.offset,
                    ap=[[Bo.ap[0][0], osb], [D, noch], [1, D]],
                ),
                in_=bass.AP(
                    tensor=xb_sb.tensor, offset=xb_sb.offset,
                    ap=[[xb_sb.ap[0][0], osb], [CB, noch], [1, D], [D, BLK]],
                ),
                axis=mybir.AxisListType.X,
                op=mybir.AluOpType.add,
            )

        # ---- main stream: tapered tiles -> block sums B ----
        B = bpool.tile([P, nchunk * D], F32)
        KCMAX = max(KCS)
        c0 = 0
        oh_next = 0
        reds = []
        for ti, KC in enumerate(KCS):
            xt = xpool.tile([P, KCMAX * CB], F32, tag="xt", name="xt")
            nc.sync.dma_start(
                out=xt[:, : KC * CB],
                in_=ap_of(
                    xw, c0 * P * CB,
                    [[CB, P], [CB * P, KC], [1, CB]],
                ),
            )
            red = nc.vector.tensor_reduce(
                out=bass.AP(
                    tensor=B.tensor, offset=B.offset + c0 * D,
                    ap=[B.ap[0], [D, KC], [1, D]],
                ),
                in_=bass.AP(
                    tensor=xt.tensor, offset=xt.offset,
                    ap=[xt.ap[0], [CB, KC], [1, D], [D, BLK]],
                ),
                axis=mybir.AxisListType.X,
                op=mybir.AluOpType.add,
            )
            c0 += KC
            reds.append(red)
            # Order the small DVE ops (one-hot builds, overflow reduce) AFTER
            # this tile's reduce so the scheduler cannot hoist them ahead of
            # the reduce pipeline (that would starve the x-DMA slot rotation),
            # and pack them into the EARLY tiles so the late tiles' reduces
            # run back-to-back (short post-DMA tail).
            if ti == min(2, len(KCS) - 1):
                tile.add_dep_helper(emit_bo_reduce().ins, red.ins, sync=False,
                                    reason="keep Bo reduce behind tile reduces")
            if ti == min(3, len(KCS) - 1):
                tile.add_dep_helper(emit_oo().ins, red.ins, sync=False,
                                    reason="keep oneO build behind tile reduces")
            quota = 2 if ti == 0 else 4 * ti + 2
            while oh_next < nstripe and (
                oh_next < quota or ti == len(KCS) - 1
            ):
                tile.add_dep_helper(emit_oh(oh_next).ins, red.ins, sync=False,
                                    reason="keep onehot build behind tile reduces")
                oh_next += 1

        # winv/bne loads: only the projection needs them, so keep the
        # 0.66 MB transfer out of the x-stream DMA window.
        wdma = nc.gpsimd.dma_start(out=winv_sb[:], in_=winv_t.ap())
        bdma = nc.gpsimd.dma_start(out=bne_sb[:], in_=bne_t.ap())
        if len(reds) >= 3:
            tile.add_dep_helper(wdma.ins, reds[-3].ins, sync=False,
                                reason="winv load off the x-stream window")
            tile.add_dep_helper(bdma.ins, reds[-3].ins, sync=False,
                                reason="bne load off the x-stream window")

        # ---- routing matmuls ----
        # Each 32-graph quadrant accumulates ALL stripes into one PSUM tile
        # [GPW, nstripe*D] (960B — fits one bank; stripe s owns columns
        # s*D..(s+1)*D). One start=True opener per quadrant clears the bank's
        # has_written bits; every other matmul accumulates-or-overwrites its
        # disjoint region, which is exact for disjoint column writes.
        nquad = P // GPW
        psums = [ppool.tile([GPW, nstripe * D], F32, name=f"ps{q}")
                 for q in range(nquad)]
        openers = [None] * nquad
        for s in range(nstripe):
            for q in range(nquad):
                psum = psums[q]
                mms = []
                for j in range(CPW):
                    m = (s * wps + q) * CPW + j
                    mms.append(nc.tensor.matmul(
                        out=psum[:, s * D : (s + 1) * D],
                        lhsT=onehot[:, m * GPW : (m + 1) * GPW],
                        rhs=B[:, m * D : (m + 1) * D],
                        start=(s == 0 and j == 0),
                        stop=(s == nstripe - 1 and j == CPW - 1),
                        tile_position=(0, 0),
                        skip_group_check=True,
                    ))
                for oc in range(plan.ocps):
                    ch = s * plan.ocps + oc
                    mms.append(nc.tensor.matmul(
                        out=psum[:, s * D : (s + 1) * D],
                        lhsT=oneO[:osb, ch * P + q * GPW : ch * P + (q + 1) * GPW],
                        rhs=Bo[:osb, ch * D : (ch + 1) * D],
                        start=False,
                        stop=False,
                        tile_position=(0, 0),
                        skip_group_check=True,
                    ))
                if s == 0:
                    openers[q] = mms[0]
                    mms = mms[1:]
                # the opener's bank-wide has_written clear must run first
                for mm in mms:
                    tile.add_dep_helper(mm.ins, openers[q].ins, sync=False,
                                        reason="psum opener first")

        # flush each quadrant once (ACT), then recombine on PE via a constant
        # identity selection matmul into a single [128, nstripe*D] PSUM tile
        sums_q = [bpool.tile([GPW, nstripe * D], F32, name=f"sumsq{q}")
                  for q in range(nquad)]
        for q in range(nquad):
            eng = nc.scalar.copy if q % 2 == 0 else nc.vector.tensor_copy
            eng(out=sums_q[q][:, :], in_=psums[q][:, :])
        psum_all = ppool.tile([P, nstripe * D], F32)
        for q in range(nquad):
            nc.tensor.matmul(
                out=psum_all[:, :],
                lhsT=i4_sb[:, q * P : (q + 1) * P],
                rhs=sums_q[q][:, :],
                start=(q == 0),
                stop=(q == nquad - 1),
                tile_position=(0, 0),
                skip_group_check=True,
            )


        # projection straight from PSUM, in two stripe-halves so the first
        # half's output DMA overlaps the second half's DVE work:
        #   tmp[p,o,s,f] = psum_all[p,s,f] * winv[p,o,s,f]
        #   proj[p,s*O+o] = sum_f tmp ;  out = proj + bne
        proj = bpool.tile([P, nstripe * O], F32)
        tmp = bpool.tile([P, O * nstripe * D], F32)
        outv = bpool.tile([P, nstripe * O], F32)
        sh = max(1, nstripe // 2)
        s0 = 0
        while s0 < nstripe:
            sn = min(sh, nstripe - s0)
            nc.vector.tensor_tensor(
                out=bass.AP(
                    tensor=tmp.tensor, offset=tmp.offset + s0 * D,
                    ap=[tmp.ap[0], [nstripe * D, O], [D, sn], [1, D]],
                ),
                in0=bass.AP(
                    tensor=psum_all.tensor, offset=psum_all.offset + s0 * D,
                    ap=[psum_all.ap[0], [0, O], [D, sn], [1, D]],
                ),
                in1=bass.AP(
                    tensor=winv_sb.tensor, offset=winv_sb.offset + s0 * D,
                    ap=[winv_sb.ap[0], [nstripe * D, O], [D, sn], [1, D]],
                ),
                op=mybir.AluOpType.mult,
            )
            nc.vector.tensor_reduce(
                out=bass.AP(
                    tensor=proj.tensor, offset=proj.offset + s0 * O,
                    ap=[proj.ap[0], [1, O], [O, sn], [1, 1]],
                ),
                in_=bass.AP(
                    tensor=tmp.tensor, offset=tmp.offset + s0 * D,
                    ap=[tmp.ap[0], [nstripe * D, O], [D, sn], [1, D]],
                ),
                axis=mybir.AxisListType.X,
                op=mybir.AluOpType.add,
            )
            # out = proj + b*nonempty  (empty graphs have exact 0 in proj)
            nc.vector.tensor_tensor(
                out=outv[:, s0 * O : (s0 + sn) * O],
                in0=proj[:, s0 * O : (s0 + sn) * O],
                in1=bne_sb[:, s0 * O : (s0 + sn) * O],
                op=mybir.AluOpType.add,
            )
            nc.sync.dma_start(
                out=ap_of(out_t, s0 * P * O, [[O, P], [P * O, sn], [1, O]]),
                in_=outv[:, s0 * O : (s0 + sn) * O],
            )
            s0 += sn

    nc.compile()
    return nc


# ----------------------------------------------------------------------------
# entry point
# ----------------------------------------------------------------------------

_CACHE = {}
_LAST_RESULTS = None


def kernel(x, batch, W, b):
    global _LAST_RESULTS
    x = np.asarray(x, dtype=np.float32)
    batch = np.asarray(batch)
    W = np.asarray(W, dtype=np.float32)
    b = np.asarray(b, dtype=np.float32)

    n_cores = 8
    G = 16384
    plan = Plan(batch, n_cores, G, W, b)

    key = (plan.lslots, plan.nchunk, plan.noch, plan.osps)
    if key not in _CACHE:
        _CACHE[key] = build_program(plan, W, b)
    nc = _CACHE[key]

    in_maps = [plan.core_tables(c, x) for c in range(n_cores)]

    def _run():
        return run_bass_kernel_spmd(nc, in_maps, core_ids=list(range(n_cores)))

    try:
        res = _run()
    except ModuleNotFoundError:
        # BASS_TRACE was set but this container lacks the axon NTFF profiling
        # hook (antenv.axon_hooks) — retry with tracing disabled.
        import os
        os.environ["BASS_NEVER_TRACE"] = "1"
        res = _run()
    except Exception as e:  # transient device/terminal failure -> one retry
        if not any(k in str(e) for k in ("UNAVAILABLE", "UNRECOVERABLE")):
            raise
        import time as _time
        _time.sleep(10.0)
        res = _run()
    _LAST_RESULTS = res
    out = np.concatenate(
        [res.results[c]["out"].reshape(plan.gpc, O) for c in range(n_cores)], axis=0
    )
    return out.astype(np.float32)


if __name__ == "__main__":
    # tiny smoke test of the planner only
    rng = np.random.default_rng(0)
    N, G = 400_000, 16384
    batch = np.sort(rng.integers(0, G, N))
    x = rng.standard_normal((N, D), dtype=np.float32)
    plan = Plan(batch, 8, G)
    print("lslots", plan.lslots, "nchunk", plan.nchunk, "osps", plan.osps)
    t = plan.core_tables(0, x)
    for k, v in t.items():
        print(k, v.shape, v.dtype)

